# revision 1
# baseline (speedup 1.0000x reference)
"""Trainium2 Bass kernel for nn_CoulombPotential (PhysNet-attenuated Coulomb energy).

Algorithm
---------
  per_system[s] = KE * sum_{pairs p: i<j, sys(i)=s} q[i] q[j] chi(d_p)
  chi(d) = phi(2d)/sqrt(d^2+1) + (1-phi(2d))/d,  phi = PhysNet switching fn.

Sharding / host marshalling (no float arithmetic on host — only data movement):
  * drop masked (i>=j) pairs, group pairs by system (sys[idx_i]; sys is sorted
    over atoms), serpentine-assign 128 systems to each of 8 cores (balanced
    by pair count; the i<j mask makes low systems ~15x heavier than high ones),
  * within a core, each system's pairs are padded to whole 512-slot rows, laid
    out as [768, 1536] f32 streams (6 tiles of [128, 1536] = 3 sub-rows of 512),
  * charge values for both endpoints are laid alongside as streams (gather is
    pure data movement), plus a 0/1 row->system selector for the PE.

Device (all arithmetic): chi pipeline on ACT+DVE, per-row sums on DVE, the
rows->systems segment reduction as 0/1-selector matmuls accumulated in PSUM on
the PE, and the final KE scale.  Core outputs are disjoint [128]-system slices;
the host only concatenates them.
"""
import functools

import numpy as np

import concourse.bacc as bacc
import concourse.bass_utils as bass_utils
import concourse.mybir as mybir
import concourse.tile as tile

F32 = mybir.dt.float32
AF = mybir.ActivationFunctionType
OP = mybir.AluOpType

KE = 138.96
N_CORES = 8
S_TOTAL = 1024
SYS_PER_CORE = S_TOTAL // N_CORES  # 128

PART = 128          # SBUF partitions
ROW = 512           # slots per logical row (system padding granularity)
T = 1536            # free dim per tile (= 3 sub-rows)
SUB = T // ROW      # sub-rows per partition per tile
TPC = 6             # tiles per core
ROWS_PER_TILE = PART * SUB          # 384 global rows per tile
ROWS_TOT = TPC * ROWS_PER_TILE      # 2304 rows per core
SLOTS = ROWS_TOT * ROW              # 1,179,648 slots per core


@functools.lru_cache(maxsize=1)
def _register_phi_op():
    """Fused DVE op: out = ((192 d - 240) d + 80) * d^3  (the PhysNet
    switching-function polynomial core; relu(1 - out) is applied on ACT).
    Registered via the documented OPS-append flow, sha pinned on the fly."""
    import concourse.dve_ops as dve_ops
    from concourse.dve_spec import Spec, Src0, sq, lower
    from concourse.dve_uop import DveOpSpec
    for o in dve_ops.OPS:
        if o.name == "PHI_COULOMB":
            return o
    body = (((Src0 * dve_ops.C0 + dve_ops.C1) * Src0 + dve_ops.C2)
            * (sq(Src0) * Src0))
    spec = Spec(body=body,
                reference=lambda in0, s0, s1, imm2:
                    (((in0 * s0 + s1) * in0 + imm2) * in0**3).astype(np.float32))
    shas = {v: DveOpSpec(name="PHI_COULOMB", opcode=1,
                         uops=lower(spec, ver=v)).sha(v) for v in ("v3", "v4")}
    op = dve_ops.DveOp("PHI_COULOMB", spec, subdim=False, uops_sha=shas)
    dve_ops.OPS.append(op)
    dve_ops.CUSTOM_DVE_SPECS[op.name] = op.spec
    dve_ops._SUB_OPCODE_FOR_NAME[op.name] = (
        dve_ops._CUSTOM_DVE_ROW_BASE + len(dve_ops.OPS) - 1)
    return op


@functools.lru_cache(maxsize=2)
def _build_nc(repeat=0):
    """repeat=0: straight-line kernel.  repeat=R>0: wrap the body in a
    hardware For_i loop running it R times (identical result; used by the
    test harness to measure per-iteration device time via slope)."""
    phi_op = _register_phi_op()
    nc = bacc.Bacc("TRN2", target_bir_lowering=False, debug=False,
                   enable_asserts=False, num_devices=N_CORES)
    d_in = nc.dram_tensor("d_in", [TPC * PART, T], F32, kind="ExternalInput")
    qi_in = nc.dram_tensor("qi_in", [TPC * PART, T], F32, kind="ExternalInput")
    qj_in = nc.dram_tensor("qj_in", [TPC * PART, T], F32, kind="ExternalInput")
    m_in = nc.dram_tensor("m_in", [TPC * PART, SUB * PART], F32,
                          kind="ExternalInput")
    out = nc.dram_tensor("out", [PART, 1], F32, kind="ExternalOutput")

    with tile.TileContext(nc) as tc:
        with (
            tc.tile_pool(name="io", bufs=2) as io,
            tc.tile_pool(name="tmp", bufs=2) as tmp,
            tc.tile_pool(name="acc", bufs=1) as acc,
            tc.tile_pool(name="psum", bufs=1, space="PSUM") as psp,
        ):
            ps = psp.tile([PART, 1], F32)

            def body():
                for t in range(TPC):
                    rs = slice(t * PART, (t + 1) * PART)
                    d = io.tile([PART, T], F32, tag="d")
                    qi = io.tile([PART, T], F32, tag="qi")
                    qj = io.tile([PART, T], F32, tag="qj")
                    mt = io.tile([PART, SUB, PART], F32, tag="mt")
                    nc.sync.dma_start(d[:], d_in[rs, :])
                    nc.sync.dma_start(qi[:], qi_in[rs, :])
                    nc.sync.dma_start(qj[:], qj_in[rs, :])
                    nc.sync.dma_start(mt[:], m_in[rs, :])

                    b1 = tmp.tile([PART, T], F32, tag="b1")
                    b2 = tmp.tile([PART, T], F32, tag="b2")
                    b3 = tmp.tile([PART, T], F32, tag="b3")
                    b4 = tmp.tile([PART, T], F32, tag="b4")
                    rsum = tmp.tile([PART, SUB], F32, tag="rsum")

                    # qq = qi*qj on the otherwise-idle GPSIMD engine
                    nc.gpsimd.tensor_tensor(b4[:], qi[:], qj[:], OP.mult)
                    # b1 = sqrt(d^2+1) ; b1 <- 1/sqrt(d^2+1) ; b2 = 1/d
                    nc.scalar.activation(b1[:], d[:], AF.Square)
                    nc.scalar.activation(b1[:], b1[:], AF.Sqrt, bias=1.0, scale=1.0)
                    nc.vector.reciprocal(b1[:], b1[:])
                    nc.vector.reciprocal(b2[:], d[:])
                    # fused poly core, then phi = relu(1 - poly) on ACT
                    nc.vector._custom_dve(phi_op, out=b3[:], in0=d[:],
                                          s0=192.0, s1=-240.0, imm2=80.0)
                    nc.scalar.activation(b3[:], b3[:], AF.Relu, bias=1.0, scale=-1.0)
                    # chi = 1/d + phi*(1/sqrt(d^2+1) - 1/d)   (in b1)
                    nc.vector.tensor_tensor(b1[:], b1[:], b2[:], OP.subtract)
                    nc.vector.tensor_tensor(b1[:], b3[:], b1[:], OP.mult)
                    nc.vector.tensor_tensor(b1[:], b1[:], b2[:], OP.add)
                    # e = qq*chi ; rowsums over the SUB sub-rows of 512
                    nc.vector.tensor_tensor(b1[:], b4[:], b1[:], OP.mult)
                    nc.vector.tensor_reduce(
                        rsum[:], b1[:].rearrange("p (s r) -> p s r", s=SUB),
                        mybir.AxisListType.X, OP.add)
                    for n in range(SUB):
                        nc.tensor.matmul(ps[:], mt[:, n, :], rsum[:, n:n + 1],
                                         start=(t == 0 and n == 0),
                                         stop=(t == TPC - 1 and n == SUB - 1))

            if repeat > 0:
                with tc.For_i(0, repeat, 1):
                    body()
            else:
                body()
            res = acc.tile([PART, 1], F32, tag="res")
            nc.scalar.mul(res[:], ps[:], KE)
            nc.sync.dma_start(out[:], res[:])
    nc.compile()
    return nc


def _host_marshal(electrostatic_pair_indices, electrostatic_d_ij,
                  per_atom_charge, atomic_subsystem_indices):
    idx_i = np.asarray(electrostatic_pair_indices[0])
    idx_j = np.asarray(electrostatic_pair_indices[1])
    d = np.asarray(electrostatic_d_ij)[:, 0]
    q = np.asarray(per_atom_charge)[:, 0].astype(np.float32)
    sys_idx = np.asarray(atomic_subsystem_indices)

    keep = idx_i < idx_j
    ii = idx_i[keep]
    jj = idx_j[keep]
    dd = d[keep].astype(np.float32)
    seg = sys_idx[ii].astype(np.int64)

    order = np.argsort(seg, kind="stable")
    ii = ii[order]
    jj = jj[order]
    dd = dd[order]
    seg = seg[order]

    counts = np.bincount(seg, minlength=S_TOTAL)
    sys_start = np.concatenate([[0], np.cumsum(counts)])

    # The i<j mask keeps more pairs for low atom indices, so per-system pair
    # counts fall roughly linearly with system id; a contiguous block split
    # is badly imbalanced.  Serpentine-assign systems (by descending count)
    # to cores: balanced within ~1% and exactly 128 systems per core.
    order_sys = np.argsort(-counts, kind="stable")
    k = np.arange(S_TOTAL)
    block, within = k // N_CORES, k % N_CORES
    core_of_rank = np.where(block % 2 == 0, within, N_CORES - 1 - within)
    sys_to_core = np.empty(S_TOTAL, np.int64)
    sys_to_core[order_sys] = core_of_rank
    # local slot of each system within its core (order of assignment)
    sys_to_local = np.empty(S_TOTAL, np.int64)
    core_systems = np.empty((N_CORES, SYS_PER_CORE), np.int64)
    for c in range(N_CORES):
        mine = order_sys[core_of_rank == c]
        core_systems[c] = mine
        sys_to_local[mine] = np.arange(SYS_PER_CORE)

    # per-core row layout: each system padded to whole 512-slot rows
    rows_of_sys = -(-counts // ROW)               # global, by system id
    core_row_base = np.empty(S_TOTAL, np.int64)   # first row of sys in its core
    n_rows_core = np.empty(N_CORES, np.int64)
    for c in range(N_CORES):
        mine = core_systems[c]
        rb = np.concatenate([[0], np.cumsum(rows_of_sys[mine])])
        core_row_base[mine] = rb[:-1]
        n_rows_core[c] = rb[-1]
    assert n_rows_core.max() <= ROWS_TOT, n_rows_core
    assert int(counts.max()) <= ROWS_TOT * ROW

    dest_core = sys_to_core[seg]
    dest_slot = core_row_base[seg] * ROW + (np.arange(len(seg)) - sys_start[seg])

    in_maps = []
    for c in range(N_CORES):
        sel = dest_core == c
        dest = dest_slot[sel]
        dstream = np.ones(SLOTS, np.float32)
        qis = np.zeros(SLOTS, np.float32)
        qjs = np.zeros(SLOTS, np.float32)
        dstream[dest] = dd[sel]
        qis[dest] = q[ii[sel]]
        qjs[dest] = q[jj[sel]]

        # 0/1 selector: global row g (slot // ROW) -> local system slot
        row_sys = np.repeat(sys_to_local[core_systems[c]],
                            rows_of_sys[core_systems[c]])
        m = np.zeros((ROWS_TOT, SYS_PER_CORE), np.float32)
        m[np.arange(n_rows_core[c]), row_sys] = 1.0
        # row g = t*512 + p*4 + n  ->  [TPC, PART, SUB, 128] -> [TPC*PART, SUB*128]
        m = m.reshape(TPC, PART, SUB, SYS_PER_CORE).reshape(TPC * PART, SUB * SYS_PER_CORE)

        in_maps.append({
            "d_in": dstream.reshape(TPC * PART, T),
            "qi_in": qis.reshape(TPC * PART, T),
            "qj_in": qjs.reshape(TPC * PART, T),
            "m_in": np.ascontiguousarray(m),
        })
    return in_maps, core_systems


def kernel(electrostatic_pair_indices, electrostatic_d_ij, per_atom_charge,
           atomic_subsystem_indices, num_systems):
    assert int(num_systems) == S_TOTAL
    in_maps, core_systems = _host_marshal(
        electrostatic_pair_indices, electrostatic_d_ij,
        per_atom_charge, atomic_subsystem_indices)
    nc = _build_nc()
    res = bass_utils.run_bass_kernel_spmd(nc, in_maps,
                                          core_ids=list(range(N_CORES)))
    full = np.empty(S_TOTAL, np.float32)
    for c in range(N_CORES):
        full[core_systems[c]] = res.results[c]["out"][:, 0]
    return full[:, None]



# revision 5
# speedup vs baseline: 5.0934x; 5.0934x over previous
"""Trainium2 Bass kernel for nn_CoulombPotential (PhysNet-attenuated Coulomb energy).

Algorithm
---------
  per_system[s] = KE * sum_{pairs p: i<j, sys(i)=s} q[i] q[j] chi(d_p)
  chi(d) = phi(2d)/sqrt(d^2+1) + (1-phi(2d))/d,  phi = PhysNet switching fn.

Key structure (v2):
  * phi(2d) == 0 exactly for d >= 0.5, so pairs split into region A (d<0.5,
    full chi pipeline, ~37.5%) and region B (d>=0.5, chi = 1/d, ~62.5%).
  * streams are fp16 (d, q_i, q_j gathered on host = pure data movement);
    all arithmetic on device.  DVE runs 2-byte ops in 2x/4x perf modes.
  * 1/d = Ars(Square(d)) and 1/sqrt(d^2+1) = Ars(Square(d)+1) with
    Ars = Abs_reciprocal_sqrt -- Square/Ars/Copy share ONE activation table,
    so the ACT engine never swaps tables.
  * phi-polynomial core runs as one fused 8-stage custom DVE op
    u = ((192d-240)d+80)d^3 * a; since a = rs - r < 0 always,
    phi*a = relu(1-poly)*a = min(a - u, 0)  (cheap 4x tensor_scalar min).
  * per-row (512-slot) sums via tensor_scalar accum_out (4x mode),
    rows->systems via 0/1-selector matmuls on the PE; selectors are built
    on device from an iota constant + row->system ids (is_equal, 4x).
  * host only: mask/gather/sort/pad/cast + final scatter of 8x[128] outputs.
"""
import functools

import numpy as np

import concourse.bacc as bacc
import concourse.bass_utils as bass_utils
import concourse.mybir as mybir
import concourse.tile as tile

F32 = mybir.dt.float32
F16 = mybir.dt.float16
AF = mybir.ActivationFunctionType
OP = mybir.AluOpType

KE = 138.96
N_CORES = 8
S_TOTAL = 1024
SYS_PER_CORE = S_TOTAL // N_CORES  # 128

PART = 128
ROW = 512            # slots per logical row (per (system,branch) padding unit)
SUB_A = 7            # A-region sub-rows per partition  -> W_A = 3584 cols
SUB_B = 11           # B-region sub-rows per partition  -> W_B = 5632 cols
W_A = SUB_A * ROW
W_B = SUB_B * ROW
R_A = PART * SUB_A   # 896 A rows per core
R_B = PART * SUB_B   # 1408 B rows per core
NG = SUB_A + SUB_B   # reduce/selector groups (one per sub-row index)

# chunking of region A for ACT->DVE pipeline overlap (columns, 512-multiples)
A_CHUNKS = (2048, 1536)
B_CHUNKS = (3072, 2560)


@functools.lru_cache(maxsize=1)
def _register_polymul_op():
    """Fused DVE op: out = (((192*Src0 - 240)*Src0 + 80) * Src0^3) * Src1.
    (s0=192, s1=-240, imm2=80 at the call site.)"""
    import concourse.dve_ops as dve_ops
    from concourse.dve_spec import Spec, Src0, Src1, sq, lower
    from concourse.dve_uop import DveOpSpec
    for o in dve_ops.OPS:
        if o.name == "POLY_COULOMB_MUL":
            return o
    body = (((Src0 * dve_ops.C0 + dve_ops.C1) * Src0 + dve_ops.C2)
            * (sq(Src0) * Src0)) * Src1
    spec = Spec(body=body,
                reference=lambda in0, in1, s0, s1, imm2:
                    ((((in0 * s0 + s1) * in0 + imm2) * in0**3) * in1
                     ).astype(np.float32))
    shas = {v: DveOpSpec(name="POLY_COULOMB_MUL", opcode=1,
                         uops=lower(spec, ver=v)).sha(v) for v in ("v3", "v4")}
    op = dve_ops.DveOp("POLY_COULOMB_MUL", spec, subdim=False, uops_sha=shas)
    dve_ops.OPS.append(op)
    dve_ops.CUSTOM_DVE_SPECS[op.name] = op.spec
    dve_ops._SUB_OPCODE_FOR_NAME[op.name] = (
        dve_ops._CUSTOM_DVE_ROW_BASE + len(dve_ops.OPS) - 1)
    return op


@functools.lru_cache(maxsize=2)
def _build_nc(repeat=0):
    """repeat=0: straight-line kernel.  repeat=R>0: wrap the body in a
    hardware For_i loop running it R times (used for slope timing)."""
    poly_op = _register_polymul_op()
    nc = bacc.Bacc("TRN2", target_bir_lowering=False, debug=False,
                   enable_asserts=False, num_devices=N_CORES)
    d_a = nc.dram_tensor("d_a", [PART, W_A], F16, kind="ExternalInput")
    qi_a = nc.dram_tensor("qi_a", [PART, W_A], F16, kind="ExternalInput")
    qj_a = nc.dram_tensor("qj_a", [PART, W_A], F16, kind="ExternalInput")
    d_b = nc.dram_tensor("d_b", [PART, W_B], F16, kind="ExternalInput")
    qi_b = nc.dram_tensor("qi_b", [PART, W_B], F16, kind="ExternalInput")
    qj_b = nc.dram_tensor("qj_b", [PART, W_B], F16, kind="ExternalInput")
    rsys_in = nc.dram_tensor("rsys_in", [PART, NG], F32, kind="ExternalInput")
    iota_in = nc.dram_tensor("iota_in", [PART, PART], F16, kind="ExternalInput")
    out = nc.dram_tensor("out", [PART, 1], F32, kind="ExternalOutput")

    with tile.TileContext(nc) as tc:
        with (
            tc.tile_pool(name="io", bufs=2) as io,
            tc.tile_pool(name="wk", bufs=1) as wk,
            tc.tile_pool(name="acc", bufs=1) as acc,
            tc.tile_pool(name="psum", bufs=1, space="PSUM") as psp,
        ):
            ps = psp.tile([PART, 1], F32)

            def body():
                # ---- input DMAs (B first: feeds the Pool engine early) ----
                dB = io.tile([PART, W_B], F16, tag="dB")
                qiB = io.tile([PART, W_B], F16, tag="qiB")
                qjB = io.tile([PART, W_B], F16, tag="qjB")
                dA = io.tile([PART, W_A], F16, tag="dA")
                qiA = io.tile([PART, W_A], F16, tag="qiA")
                qjA = io.tile([PART, W_A], F16, tag="qjA")
                rsys = io.tile([PART, NG], F32, tag="rsys")
                iota = io.tile([PART, PART], F16, tag="iota")
                nc.sync.dma_start(dA[:], d_a[:, :])
                nc.sync.dma_start(qiB[:], qi_b[:, :])
                nc.sync.dma_start(qjB[:], qj_b[:, :])
                nc.sync.dma_start(dB[:], d_b[:, :])
                nc.sync.dma_start(qiA[:], qi_a[:, :])
                nc.sync.dma_start(qjA[:], qj_a[:, :])
                nc.sync.dma_start(rsys[:], rsys_in[:, :])
                nc.sync.dma_start(iota[:], iota_in[:, :])

                # ---- work buffers ----
                qqA = wk.tile([PART, W_A], F16, tag="qqA")
                s2A = wk.tile([PART, W_A], F16, tag="s2A")
                rsA = wk.tile([PART, W_A], F16, tag="rsA")
                rA = wk.tile([PART, W_A], F16, tag="rA")
                qqB = wk.tile([PART, W_B], F16, tag="qqB")
                s2B = wk.tile([PART, W_B], F16, tag="s2B")
                rsum = wk.tile([PART, NG], F32, tag="rsum")
                rsum16 = wk.tile([PART, NG], F16, tag="rsum16")
                red = wk.tile([PART, ROW], F16, tag="red")

                # ---- region B: chi = 1/d ----
                # Pool: qq (software engine, one big instr)
                nc.gpsimd.tensor_tensor(qqB[:], qiB[:], qjB[:], OP.mult)
                # ACT: r = Ars(d^2) in-place over s2
                nc.scalar.activation(s2B[:], dB[:], AF.Square)
                nc.scalar.activation(s2B[:], s2B[:], AF.Abs_reciprocal_sqrt)

                # ---- region A: full chi, chunked for pipeline overlap ----
                qqA_on_pool = True
                if qqA_on_pool:
                    nc.gpsimd.tensor_tensor(qqA[:], qiA[:], qjA[:], OP.mult)
                off = 0
                for w in A_CHUNKS:
                    cs = slice(off, off + w)
                    off += w
                    if not qqA_on_pool:
                        nc.vector.tensor_tensor(qqA[:, cs], qiA[:, cs], qjA[:, cs],
                                                OP.mult)
                    # ACT: s2 = d^2 ; rs = Ars(s2+1) ; r = Ars(s2)
                    nc.scalar.activation(s2A[:, cs], dA[:, cs], AF.Square)
                    nc.scalar.activation(rsA[:, cs], s2A[:, cs],
                                         AF.Abs_reciprocal_sqrt, bias=1.0)
                    nc.scalar.activation(rA[:, cs], s2A[:, cs],
                                         AF.Abs_reciprocal_sqrt)
                    # DVE: a = rs - r (<0); u = poly*a ; b = min(a-u, 0);
                    #      chi = b + r
                    nc.vector.tensor_tensor(rsA[:, cs], rsA[:, cs], rA[:, cs],
                                            OP.subtract)       # a  (in rsA)
                    nc.vector._custom_dve(poly_op, out=s2A[:, cs], in0=dA[:, cs],
                                          in1=rsA[:, cs],
                                          s0=192.0, s1=-240.0, imm2=80.0)
                    nc.vector.tensor_tensor(rsA[:, cs], rsA[:, cs], s2A[:, cs],
                                            OP.subtract)       # a-u (in rsA)
                    nc.vector.tensor_scalar(rsA[:, cs], rsA[:, cs], 0.0, None,
                                            OP.min)            # b
                    nc.vector.tensor_tensor(rA[:, cs], rsA[:, cs], rA[:, cs],
                                            OP.add)            # chi (in rA)
                    # DVE: e = qq*chi (in qqA)
                    nc.vector.tensor_tensor(qqA[:, cs], qqA[:, cs], rA[:, cs], OP.mult)

                # DVE: e_B = qq*r (in qqB)
                nc.vector.tensor_tensor(qqB[:], qqB[:], s2B[:], OP.mult)

                # ---- per-sub-row sums (tensor_scalar accum, 4x mode) ----
                for g in range(SUB_A):
                    nc.vector.tensor_scalar(
                        red[:], qqA[:, g * ROW:(g + 1) * ROW], 1.0, 0.0,
                        OP.mult, OP.add, accum_out=rsum[:, g:g + 1])
                for g in range(SUB_B):
                    nc.vector.tensor_scalar(
                        red[:], qqB[:, g * ROW:(g + 1) * ROW], 1.0, 0.0,
                        OP.mult, OP.add, accum_out=rsum[:, SUB_A + g:SUB_A + g + 1])
                nc.vector.tensor_copy(rsum16[:], rsum[:])

                # ---- rows -> systems selector matmuls on the PE ----
                for g in range(NG):
                    sel = io.tile([PART, PART], F16, tag="sel")
                    nc.vector.tensor_scalar(sel[:], iota[:],
                                            rsys[:, g:g + 1], None,
                                            OP.is_equal)
                    nc.tensor.matmul(ps[:], sel[:], rsum16[:, g:g + 1],
                                     start=(g == 0), stop=(g == NG - 1))

            if repeat > 0:
                with tc.For_i(0, repeat, 1):
                    body()
            else:
                body()
            res = acc.tile([PART, 1], F32, tag="res")
            nc.vector.tensor_scalar(res[:], ps[:], float(KE), None, OP.mult)
            nc.sync.dma_start(out[:], res[:])
    nc.compile()
    return nc


def _host_marshal(electrostatic_pair_indices, electrostatic_d_ij,
                  per_atom_charge, atomic_subsystem_indices):
    idx_i = np.asarray(electrostatic_pair_indices[0])
    idx_j = np.asarray(electrostatic_pair_indices[1])
    d = np.asarray(electrostatic_d_ij)[:, 0]
    q = np.asarray(per_atom_charge)[:, 0].astype(np.float32)
    sys_idx = np.asarray(atomic_subsystem_indices)

    keep = idx_i < idx_j
    ii = idx_i[keep]
    jj = idx_j[keep]
    dd = d[keep].astype(np.float32)
    seg = sys_idx[ii].astype(np.int64)
    br = (dd >= 0.5).astype(np.int64)        # 0 = region A, 1 = region B

    # sort by (system, branch); stable keeps determinism
    order = np.argsort(seg * 2 + br, kind="stable")
    ii = ii[order]
    jj = jj[order]
    dd = dd[order]
    seg = seg[order]
    br = br[order]

    key = seg * 2 + br                        # run id in [0, 2*S)
    counts2 = np.bincount(key, minlength=2 * S_TOTAL)  # per (sys,branch)
    run_start = np.concatenate([[0], np.cumsum(counts2)])
    counts = counts2[0::2] + counts2[1::2]    # per system

    # serpentine-assign systems to cores balanced by kept-pair count
    order_sys = np.argsort(-counts, kind="stable")
    k = np.arange(S_TOTAL)
    block, within = k // N_CORES, k % N_CORES
    core_of_rank = np.where(block % 2 == 0, within, N_CORES - 1 - within)
    sys_to_core = np.empty(S_TOTAL, np.int64)
    sys_to_core[order_sys] = core_of_rank
    sys_to_local = np.empty(S_TOTAL, np.int64)
    core_systems = np.empty((N_CORES, SYS_PER_CORE), np.int64)
    for c in range(N_CORES):
        mine = order_sys[core_of_rank == c]
        core_systems[c] = mine
        sys_to_local[mine] = np.arange(SYS_PER_CORE)

    # per-core, per-region row layout: each (sys,branch) run padded to rows
    rows_of_run = -(-counts2 // ROW)          # [2*S]
    run_row_base = np.empty(2 * S_TOTAL, np.int64)
    n_rows = np.zeros((N_CORES, 2), np.int64)
    for c in range(N_CORES):
        mine = core_systems[c]
        for b, r_cap in ((0, R_A), (1, R_B)):
            runs = mine * 2 + b
            rb = np.concatenate([[0], np.cumsum(rows_of_run[runs])])
            run_row_base[runs] = rb[:-1]
            n_rows[c, b] = rb[-1]
            assert rb[-1] <= r_cap, (c, b, rb[-1], r_cap)

    dest_core = sys_to_core[seg]
    # slot within the run, then within the region
    slot_in_run = np.arange(len(seg)) - run_start[key]
    dest_row = run_row_base[key] + slot_in_run // ROW
    dest_off = slot_in_run % ROW

    # row -> (partition, sub-row) -> flat stream index
    sub_of = np.where(br == 0, SUB_A, SUB_B)
    p_of = dest_row // sub_of
    n_of = dest_row % sub_of
    flat = p_of * np.where(br == 0, W_A, W_B) + n_of * ROW + dest_off

    qi_v = q[ii].astype(np.float16)
    qj_v = q[jj].astype(np.float16)
    dd16 = dd.astype(np.float16)

    iota_const = np.tile(np.arange(PART, dtype=np.float16), (PART, 1))

    in_maps = []
    for c in range(N_CORES):
        m = {}
        for b, W, sub, nm in ((0, W_A, SUB_A, "a"), (1, W_B, SUB_B, "b")):
            selp = (dest_core == c) & (br == b)
            f = flat[selp]
            ds = np.ones(PART * W, np.float16)
            qis = np.zeros(PART * W, np.float16)
            qjs = np.zeros(PART * W, np.float16)
            ds[f] = dd16[selp]
            qis[f] = qi_v[selp]
            qjs[f] = qj_v[selp]
            m["d_" + nm] = ds.reshape(PART, W)
            m["qi_" + nm] = qis.reshape(PART, W)
            m["qj_" + nm] = qjs.reshape(PART, W)

        # row -> local system map, per (partition, group)
        rsys = np.zeros((PART, NG), np.float32)
        for b, sub, r_cap, g0 in ((0, SUB_A, R_A, 0), (1, SUB_B, R_B, SUB_A)):
            row_sys = np.zeros(r_cap, np.int64)
            mine = core_systems[c]
            runs = mine * 2 + b
            nrows_runs = rows_of_run[runs]
            row_sys[:n_rows[c, b]] = np.repeat(sys_to_local[mine], nrows_runs)
            # row = p*sub + n  ->  rsys[p, g0+n]
            rs2 = row_sys.reshape(PART, sub)
            rsys[:, g0:g0 + sub] = rs2.astype(np.float32)
        m["rsys_in"] = rsys
        m["iota_in"] = iota_const
        in_maps.append(m)
    return in_maps, core_systems


def kernel(electrostatic_pair_indices, electrostatic_d_ij, per_atom_charge,
           atomic_subsystem_indices, num_systems):
    assert int(num_systems) == S_TOTAL
    in_maps, core_systems = _host_marshal(
        electrostatic_pair_indices, electrostatic_d_ij,
        per_atom_charge, atomic_subsystem_indices)
    nc = _build_nc()
    res = bass_utils.run_bass_kernel_spmd(nc, in_maps,
                                          core_ids=list(range(N_CORES)))
    full = np.empty(S_TOTAL, np.float32)
    for c in range(N_CORES):
        full[core_systems[c]] = res.results[c]["out"][:, 0]
    return full[:, None]
